# revision 17
# baseline (speedup 1.0000x reference)
"""Trainium2 Bass kernel for nn_CustomModel_7378753814838.

Math (reference):
    a = x1.reshape(N,R,F); b = x2.reshape(N,R,F)
    d2[k,n,i,j] = ||a[n,i] - b[n,j] - m_k||^2
    kv = exp(-d2 / (2*sigma_k^2))
    out = sum_k w_k * softmax_j(kv[k])           w = softmax(1/sigma_params^2)

Key identities used (per kernel k, all in PSUM accumulation):
    ATs = -2*(A - m_k)^T  (bf16, via PE "transpose" matmul + evac)
    BT  = B^T             (bf16)
    sqA = ATs*ATs, sqB = BT*BT   (GPSIMD elementwise)
    psum[i,j] = sum_f [ ATs[f,i]*BT[f,j] + 0.25*sqA[f,i] + sqB[f,j] ]
              = sum_f (0.5*ATs[f,i] + BT[f,j])^2  =  ||(a_i - m) - b_j||^2 = d2
    via three matmul groups: lhsT=ATs_n/rhs=BT_n ; lhsT=sqA_n/rhs=0.25-matrix ;
    lhsT=ones-matrix/rhs=sqB_group.  Then kv = exp(SCALE*psum) batched on ACT,
    E = exp(kv), softmax denom by row-reduce, combine with w_k/s.
    Kernels with negligible weight w_k (< 1e-12) are dropped host-side.

Sharding: data-parallel over N across 8 cores (16 samples each).
"""

import numpy as np

N, R, F, K = 128, 128, 128, 4
NCORES = 8
NP = N // NCORES  # samples per core


def _bf16():
    import ml_dtypes

    return ml_dtypes.bfloat16


def _patch_ldw_opt():
    import concourse.bass_utils as bu

    if getattr(bu, "_ldw_patched", False):
        return
    orig = bu.run_command

    def rc(argv, **kw):
        argv = [
            "--enable-ldw-opt=true" if a == "--enable-ldw-opt=false" else a
            for a in argv
        ]
        return orig(argv, **kw)

    bu.run_command = rc
    bu._ldw_patched = True


def _build_nc(sigmas, means, sigma_params):
    from contextlib import ExitStack

    import concourse.bacc as bacc
    import concourse.tile as tile
    from concourse import mybir

    f32 = mybir.dt.float32
    bf16 = mybir.dt.bfloat16
    ALU = mybir.AluOpType
    ACTF = mybir.ActivationFunctionType

    # ---- host-side scalar math (f64) ----
    sig = np.asarray(sigmas, dtype=np.float64)
    mu = np.asarray(means, dtype=np.float64)
    sp = np.asarray(sigma_params, dtype=np.float64)
    logits = 1.0 / (sp * sp)
    e = np.exp(logits - logits.max())
    w = e / e.sum()
    KS = [k for k in range(K) if w[k] > 1e-12]
    SCALE = [-1.0 / (2.0 * sig[k] * sig[k]) for k in range(K)]

    nc = bacc.Bacc(
        "TRN2",
        target_bir_lowering=False,
        debug=False,
        enable_asserts=False,
        num_devices=NCORES,
    )
    x1 = nc.dram_tensor("x1", [NP, R * F], f32, kind="ExternalInput").ap()
    x2 = nc.dram_tensor("x2", [NP, R * F], f32, kind="ExternalInput").ap()
    y = nc.dram_tensor("y", [NP, R, R], f32, kind="ExternalOutput").ap()

    id_p1_d = nc.inline_tensor(np.eye(R).astype(np.float32), name="id_p1").ap()
    id_m2_d = nc.inline_tensor(
        (np.eye(R) * -2.0).astype(np.float32), name="id_m2"
    ).ap()
    qmat_d = nc.inline_tensor(
        np.full((R, R), 0.25, dtype=_bf16()), name="qmat"
    ).ap()
    omat_d = nc.inline_tensor(np.ones((R, R), dtype=_bf16()), name="omat").ap()

    A_src = x1.rearrange("n (i f) -> i n f", i=R)  # [128, NP, 128]
    B_src = x2.rearrange("n (j f) -> j n f", j=R)
    y_dst = y.rearrange("n i j -> i n j")  # [128, NP, 128]

    NG = NP // 4  # groups of 4 samples

    with ExitStack() as ctx:
        tc = ctx.enter_context(tile.TileContext(nc))
        singles = ctx.enter_context(tc.tile_pool(name="singles", bufs=1))
        bigs = ctx.enter_context(tc.tile_pool(name="bigs", bufs=1))
        kbig = ctx.enter_context(tc.tile_pool(name="kbig", bufs=1))
        trash = ctx.enter_context(tc.tile_pool(name="trash", bufs=3))
        psA = ctx.enter_context(tc.tile_pool(name="psA", bufs=2, space="PSUM"))
        psB = ctx.enter_context(tc.tile_pool(name="psB", bufs=2, space="PSUM"))
        psG = ctx.enter_context(tc.tile_pool(name="psG", bufs=3, space="PSUM"))

        # constants
        id_p1 = singles.tile([R, R], f32)
        nc.sync.dma_start(id_p1[:], id_p1_d)
        id_m2 = singles.tile([R, R], f32)
        nc.sync.dma_start(id_m2[:], id_m2_d)
        qmat = singles.tile([R, R], bf16)
        nc.sync.dma_start(qmat[:], qmat_d)
        omat = singles.tile([R, R], bf16)
        nc.sync.dma_start(omat[:], omat_d)

        # inputs, 4-sample chunks for pipelining
        A = bigs.tile([R, NP, F], f32, tag="A")
        B = bigs.tile([R, NP, F], f32, tag="B")
        for g in range(NG):
            s = slice(4 * g, 4 * g + 4)
            nc.sync.dma_start(A[:, s, :], A_src[:, s, :])
            nc.sync.dma_start(B[:, s, :], B_src[:, s, :])

        BT = bigs.tile([R, NP, F], bf16, tag="BT")
        sqB = bigs.tile([R, NP, F], bf16, tag="sqB")
        ATs = {
            k: kbig.tile([R, NP, F], bf16, tag=f"ATs{k}", name=f"ATs{k}") for k in KS
        }
        sqA = {
            k: kbig.tile([R, NP, F], bf16, tag=f"sqA{k}", name=f"sqA{k}") for k in KS
        }

        OUT = bigs.tile([R, NP, F], f32, tag="OUT")
        for g in range(NG):
            s = slice(4 * g, 4 * g + 4)
            # --- transposes via normal matmul (values used; -2 baked in id_m2)
            pA = psA.tile([R, 4, F], f32, tag="pA")
            pB = psB.tile([R, 4, F], f32, tag="pB")
            for q in range(4):
                nc.tensor.matmul(
                    pA[:, q, :],
                    lhsT=A[:, 4 * g + q, :],
                    rhs=id_m2[:],
                    start=True,
                    stop=True,
                )
                nc.tensor.matmul(
                    pB[:, q, :],
                    lhsT=B[:, 4 * g + q, :],
                    rhs=id_p1[:],
                    start=True,
                    stop=True,
                )
            nc.scalar.copy(BT[:, s, :], pB[:])
            for k in KS:
                # ATs = (-2*A^T) + 2m = -2*(A-m)^T   (bf16)
                nc.vector.tensor_scalar(
                    ATs[k][:, s, :], pA[:], 2.0 * float(mu[k]), None, op0=ALU.add
                )
            # --- squares (GPSIMD, bf16) ---
            nc.gpsimd.tensor_mul(sqB[:, s, :], BT[:, s, :], BT[:, s, :])
            for k in KS:
                nc.gpsimd.tensor_mul(
                    sqA[k][:, s, :], ATs[k][:, s, :], ATs[k][:, s, :]
                )

        # ---- per-kernel main pipeline (fully per-group) ----
        for ki, k in enumerate(KS):
            sc = float(SCALE[k])
            KV = kbig.tile([R, NP, F], f32, tag="KV")
            E = kbig.tile([R, NP, F], f32, tag="E")
            last = ki == len(KS) - 1
            for g in range(NG):
                s = slice(4 * g, 4 * g + 4)
                pG = psG.tile([R, 4, F], f32, tag="pG")
                for q in range(4):
                    n = 4 * g + q
                    # -2dot' ; q==0 clears the whole bank's has_written bits
                    nc.tensor.matmul(
                        pG[:, q, :],
                        lhsT=ATs[k][:, n, :],
                        rhs=BT[:, n, :],
                        start=(q == 0),
                        stop=False,
                    )
                for q in range(4):
                    n = 4 * g + q
                    # + sa'2[i] = 0.25*sum_f sqA  (j-broadcast via 0.25-matrix)
                    nc.tensor.matmul(
                        pG[:, q, :],
                        lhsT=sqA[k][:, n, :],
                        rhs=qmat[:],
                        start=False,
                        stop=False,
                    )
                # + sb2[j] for all 4 samples: lhsT = all-ones matrix
                nc.tensor.matmul(
                    pG[:, :, :],
                    lhsT=omat[:],
                    rhs=sqB[:, s, :],
                    start=False,
                    stop=True,
                )
                nc.scalar.activation(KV[:, s, :], pG[:, :, :], ACTF.Exp, scale=sc)
                nc.scalar.activation(E[:, s, :], KV[:, s, :], ACTF.Exp)
                scol = trash.tile([R, 4], f32, tag="scol")
                nc.vector.tensor_reduce(
                    scol[:], E[:, s, :], axis=mybir.AxisListType.X, op=ALU.add
                )
                qcol = trash.tile([R, 4], f32, tag="qcol")
                nc.vector.reciprocal(qcol[:], scol[:])
                if w[k] != 1.0:
                    nc.vector.tensor_scalar(
                        qcol[:], qcol[:], float(w[k]), None, op0=ALU.mult
                    )
                for q in range(4):
                    n = 4 * g + q
                    eng = nc.vector if q < 2 else nc.gpsimd
                    if ki == 0:
                        eng.tensor_scalar(
                            OUT[:, n, :],
                            E[:, n, :],
                            qcol[:, q : q + 1],
                            None,
                            op0=ALU.mult,
                        )
                    else:
                        eng.scalar_tensor_tensor(
                            OUT[:, n, :],
                            E[:, n, :],
                            qcol[:, q : q + 1],
                            OUT[:, n, :],
                            op0=ALU.mult,
                            op1=ALU.add,
                        )
                if last:
                    nc.scalar.dma_start(y_dst[:, s, :], OUT[:, s, :])

    nc.compile()
    return nc


_CACHE = {}


def _get_nc(key, sigmas, means, sigma_params):
    if key not in _CACHE:
        _CACHE[key] = _build_nc(sigmas, means, sigma_params)
    return _CACHE[key]


def run(x1, x2, sigmas, means, sigma_params, trace=False, **rk):
    from concourse.bass_utils import run_bass_kernel_spmd

    key = (sigmas.tobytes(), means.tobytes(), sigma_params.tobytes())
    nc = _get_nc(key, sigmas, means, sigma_params)

    x1 = np.ascontiguousarray(x1, dtype=np.float32)
    x2 = np.ascontiguousarray(x2, dtype=np.float32)
    in_maps = []
    for c in range(NCORES):
        s = slice(c * NP, (c + 1) * NP)
        in_maps.append({"x1": x1[s], "x2": x2[s]})
    res = run_bass_kernel_spmd(
        nc, in_maps, core_ids=list(range(NCORES)), trace=trace, **rk
    )
    out = np.concatenate([r["y"] for r in res.results], axis=0)
    return out, res


def kernel(x1, x2, sigmas, means, sigma_params):
    out, _ = run(x1, x2, sigmas, means, sigma_params, trace=False)
    return out


# revision 18
# speedup vs baseline: 1.3130x; 1.3130x over previous
"""Trainium2 Bass kernel for nn_CustomModel_7378753814838.

Math (reference):
    a = x1.reshape(N,R,F); b = x2.reshape(N,R,F)
    d2[k,n,i,j] = ||a[n,i] - b[n,j] - m_k||^2
    kv = exp(-d2 / (2*sigma_k^2))
    out = sum_k w_k * softmax_j(kv[k])           w = softmax(1/sigma_params^2)

Key identities used (per kernel k, all in PSUM accumulation):
    ATs = -2*(A - m_k)^T  (bf16, via PE "transpose" matmul + evac)
    BT  = B^T             (bf16)
    sqA = ATs*ATs, sqB = BT*BT   (GPSIMD elementwise)
    psum[i,j] = sum_f [ ATs[f,i]*BT[f,j] + 0.25*sqA[f,i] + sqB[f,j] ]
              = sum_f (0.5*ATs[f,i] + BT[f,j])^2  =  ||(a_i - m) - b_j||^2 = d2
    via three matmul groups: lhsT=ATs_n/rhs=BT_n ; lhsT=sqA_n/rhs=0.25-matrix ;
    lhsT=ones-matrix/rhs=sqB_group.  Then kv = exp(SCALE*psum) batched on ACT,
    E = exp(kv), softmax denom by row-reduce, combine with w_k/s.
    Kernels with negligible weight w_k (< 1e-12) are dropped host-side.

Sharding: data-parallel over N across 8 cores (16 samples each).
"""

import numpy as np

N, R, F, K = 128, 128, 128, 4
NCORES = 8
NP = N // NCORES  # samples per core


def _bf16():
    import ml_dtypes

    return ml_dtypes.bfloat16


def _patch_ldw_opt():
    import concourse.bass_utils as bu

    if getattr(bu, "_ldw_patched", False):
        return
    orig = bu.run_command

    def rc(argv, **kw):
        argv = [
            "--enable-ldw-opt=true" if a == "--enable-ldw-opt=false" else a
            for a in argv
        ]
        return orig(argv, **kw)

    bu.run_command = rc
    bu._ldw_patched = True


def _build_nc(sigmas, means, sigma_params):
    from contextlib import ExitStack

    import concourse.bacc as bacc
    import concourse.tile as tile
    from concourse import mybir

    f32 = mybir.dt.float32
    bf16 = mybir.dt.bfloat16
    ALU = mybir.AluOpType
    ACTF = mybir.ActivationFunctionType

    # ---- host-side scalar math (f64) ----
    sig = np.asarray(sigmas, dtype=np.float64)
    mu = np.asarray(means, dtype=np.float64)
    sp = np.asarray(sigma_params, dtype=np.float64)
    logits = 1.0 / (sp * sp)
    e = np.exp(logits - logits.max())
    w = e / e.sum()
    KS = [k for k in range(K) if w[k] > 1e-12]
    SCALE = [-1.0 / (2.0 * sig[k] * sig[k]) for k in range(K)]

    nc = bacc.Bacc(
        "TRN2",
        target_bir_lowering=False,
        debug=False,
        enable_asserts=False,
        num_devices=NCORES,
    )
    x1 = nc.dram_tensor("x1", [NP, R * F], f32, kind="ExternalInput").ap()
    x2 = nc.dram_tensor("x2", [NP, R * F], f32, kind="ExternalInput").ap()
    y = nc.dram_tensor("y", [NP, R, R], f32, kind="ExternalOutput").ap()

    id_p1_d = nc.inline_tensor(np.eye(R).astype(np.float32), name="id_p1").ap()
    id_m2_d = nc.inline_tensor(
        (np.eye(R) * -2.0).astype(np.float32), name="id_m2"
    ).ap()
    qmat_d = nc.inline_tensor(
        np.full((R, R), 0.25, dtype=_bf16()), name="qmat"
    ).ap()
    omat_d = nc.inline_tensor(np.ones((R, R), dtype=_bf16()), name="omat").ap()

    A_src = x1.rearrange("n (i f) -> i n f", i=R)  # [128, NP, 128]
    B_src = x2.rearrange("n (j f) -> j n f", j=R)
    y_dst = y.rearrange("n i j -> i n j")  # [128, NP, 128]

    NG = NP // 4  # groups of 4 samples

    with ExitStack() as ctx:
        tc = ctx.enter_context(tile.TileContext(nc))
        singles = ctx.enter_context(tc.tile_pool(name="singles", bufs=1))
        bigs = ctx.enter_context(tc.tile_pool(name="bigs", bufs=1))
        kbig = ctx.enter_context(tc.tile_pool(name="kbig", bufs=1))
        trash = ctx.enter_context(tc.tile_pool(name="trash", bufs=3))
        psA = ctx.enter_context(tc.tile_pool(name="psA", bufs=2, space="PSUM"))
        psB = ctx.enter_context(tc.tile_pool(name="psB", bufs=2, space="PSUM"))
        psG = ctx.enter_context(tc.tile_pool(name="psG", bufs=3, space="PSUM"))

        # constants
        id_p1 = singles.tile([R, R], f32)
        nc.sync.dma_start(id_p1[:], id_p1_d)
        id_m2 = singles.tile([R, R], f32)
        nc.sync.dma_start(id_m2[:], id_m2_d)
        qmat = singles.tile([R, R], bf16)
        nc.sync.dma_start(qmat[:], qmat_d)
        omat = singles.tile([R, R], bf16)
        nc.sync.dma_start(omat[:], omat_d)

        # inputs, 4-sample chunks for pipelining
        A = bigs.tile([R, NP, F], f32, tag="A")
        B = bigs.tile([R, NP, F], f32, tag="B")
        for g in range(NG):
            s = slice(4 * g, 4 * g + 4)
            nc.sync.dma_start(A[:, s, :], A_src[:, s, :])
            nc.sync.dma_start(B[:, s, :], B_src[:, s, :])

        BT = bigs.tile([R, NP, F], bf16, tag="BT")
        sqB = bigs.tile([R, NP, F], bf16, tag="sqB")
        ATs = {
            k: kbig.tile([R, NP, F], bf16, tag=f"ATs{k}", name=f"ATs{k}") for k in KS
        }
        sqA = {
            k: kbig.tile([R, NP, F], bf16, tag=f"sqA{k}", name=f"sqA{k}") for k in KS
        }

        OUT = bigs.tile([R, NP, F], f32, tag="OUT")
        for g in range(NG):
            s = slice(4 * g, 4 * g + 4)
            # --- transposes via normal matmul (values used; -2 baked in id_m2)
            pA = psA.tile([R, 4, F], f32, tag="pA")
            pB = psB.tile([R, 4, F], f32, tag="pB")
            for q in range(4):
                nc.tensor.matmul(
                    pA[:, q, :],
                    lhsT=A[:, 4 * g + q, :],
                    rhs=id_m2[:],
                    start=True,
                    stop=True,
                )
                nc.tensor.matmul(
                    pB[:, q, :],
                    lhsT=B[:, 4 * g + q, :],
                    rhs=id_p1[:],
                    start=True,
                    stop=True,
                )
            nc.scalar.copy(BT[:, s, :], pB[:])
            for k in KS:
                # ATs = (-2*A^T) + 2m = -2*(A-m)^T   (bf16)
                nc.vector.tensor_scalar(
                    ATs[k][:, s, :], pA[:], 2.0 * float(mu[k]), None, op0=ALU.add
                )
            # --- squares (GPSIMD, bf16) ---
            nc.gpsimd.tensor_mul(sqB[:, s, :], BT[:, s, :], BT[:, s, :])
            for k in KS:
                nc.gpsimd.tensor_mul(
                    sqA[k][:, s, :], ATs[k][:, s, :], ATs[k][:, s, :]
                )

        # ---- per-kernel main pipeline (fully per-group) ----
        for ki, k in enumerate(KS):
            sc = float(SCALE[k])
            KV = kbig.tile([R, NP, F], f32, tag="KV")
            E = kbig.tile([R, NP, F], f32, tag="E")
            last = ki == len(KS) - 1
            for g in range(NG):
                s = slice(4 * g, 4 * g + 4)
                pG = psG.tile([R, 4, F], f32, tag="pG")
                for q in range(4):
                    n = 4 * g + q
                    # -2dot' ; q==0 clears the whole bank's has_written bits
                    nc.tensor.matmul(
                        pG[:, q, :],
                        lhsT=ATs[k][:, n, :],
                        rhs=BT[:, n, :],
                        start=(q == 0),
                        stop=False,
                    )
                for q in range(4):
                    n = 4 * g + q
                    # + sa'2[i] = 0.25*sum_f sqA  (j-broadcast via 0.25-matrix)
                    nc.tensor.matmul(
                        pG[:, q, :],
                        lhsT=sqA[k][:, n, :],
                        rhs=qmat[:],
                        start=False,
                        stop=False,
                    )
                # + sb2[j] for all 4 samples: lhsT = all-ones matrix
                nc.tensor.matmul(
                    pG[:, :, :],
                    lhsT=omat[:],
                    rhs=sqB[:, s, :],
                    start=False,
                    stop=True,
                )
                nc.scalar.activation(KV[:, s, :], pG[:, :, :], ACTF.Exp, scale=sc)
                nc.scalar.activation(E[:, s, :], KV[:, s, :], ACTF.Exp)
                scol = trash.tile([R, 4], f32, tag="scol")
                nc.vector.tensor_reduce(
                    scol[:], E[:, s, :], axis=mybir.AxisListType.X, op=ALU.add
                )
                qcol = trash.tile([R, 4], f32, tag="qcol")
                nc.vector.reciprocal(qcol[:], scol[:])
                if w[k] != 1.0:
                    nc.vector.tensor_scalar(
                        qcol[:], qcol[:], float(w[k]), None, op0=ALU.mult
                    )
                for q in range(4):
                    n = 4 * g + q
                    eng = nc.vector
                    if ki == 0:
                        eng.tensor_scalar(
                            OUT[:, n, :],
                            E[:, n, :],
                            qcol[:, q : q + 1],
                            None,
                            op0=ALU.mult,
                        )
                    else:
                        eng.scalar_tensor_tensor(
                            OUT[:, n, :],
                            E[:, n, :],
                            qcol[:, q : q + 1],
                            OUT[:, n, :],
                            op0=ALU.mult,
                            op1=ALU.add,
                        )
                if last:
                    nc.scalar.dma_start(y_dst[:, s, :], OUT[:, s, :])

    nc.compile()
    return nc


_CACHE = {}


def _get_nc(key, sigmas, means, sigma_params):
    if key not in _CACHE:
        _CACHE[key] = _build_nc(sigmas, means, sigma_params)
    return _CACHE[key]


def run(x1, x2, sigmas, means, sigma_params, trace=False, **rk):
    from concourse.bass_utils import run_bass_kernel_spmd

    key = (sigmas.tobytes(), means.tobytes(), sigma_params.tobytes())
    nc = _get_nc(key, sigmas, means, sigma_params)

    x1 = np.ascontiguousarray(x1, dtype=np.float32)
    x2 = np.ascontiguousarray(x2, dtype=np.float32)
    in_maps = []
    for c in range(NCORES):
        s = slice(c * NP, (c + 1) * NP)
        in_maps.append({"x1": x1[s], "x2": x2[s]})
    res = run_bass_kernel_spmd(
        nc, in_maps, core_ids=list(range(NCORES)), trace=trace, **rk
    )
    out = np.concatenate([r["y"] for r in res.results], axis=0)
    return out, res


def kernel(x1, x2, sigmas, means, sigma_params):
    out, _ = run(x1, x2, sigmas, means, sigma_params, trace=False)
    return out
